# revision 2
# baseline (speedup 1.0000x reference)
"""Grouped GEMM (MoE expert layer) on 8 Trainium2 NeuronCores.

Problem: out[t] = input[t] @ weight[expert(t)].T + bias[expert(t)], where
tokens are pre-sorted by expert and group sizes come from expert_frequency
(host-readable static metadata, same as the reference's .tolist()).

Strategy (single uniform SPMD program, all-to-all token routing on host):
  - One shared "slot profile" P: every core runs S slots; slot s processes
    P[s] tiles of 128 tokens with one weight matrix. Slot weights/biases and
    the token blocks are per-core DATA (host-gathered), so one NEFF serves
    all 8 cores despite the uneven expert sizes.
  - A planner packs (expert, tile-range) pieces into the 8xP global slot
    inventory with ~2% padding at C = ceil(total_tiles/8).
  - Matmul layout: W-stationary. lhsT = WT[kc, dc-chunk] (128x128),
    moving = XT[kc, 512-token block], psum = [128 dout, 512 tok]. 4 psum
    banks interleaved per 2048-token megatile to pipeline the PE.
  - fp16 operands (PE runs fp16 at full rate; fp32 runs 4x slower and f32r
    2x slower due to the in-instruction weight-load). PSUM accumulation is
    fp32; outputs written fp32. End-to-end rel error ~3e-4 vs fp32.
  - Input X is transposed on host ([d_in, tokens] fp16) so every device DMA
    is contiguous-row; output is produced transposed ([d_out, tokens] fp32)
    and transposed back on host.
"""

import numpy as np

import concourse.bacc as bacc
import concourse.mybir as mybir
import concourse.tile as tile
from concourse.bass_utils import run_bass_kernel_spmd

N_CORES = 8
KC = 8          # contraction chunks (d_in = KC*128)
DC = 8          # d_out chunks (d_out = DC*128)
D_IN = 1024
D_OUT = 1024
TILE = 128
MEGA_TOK = 2048     # tokens per megatile (4 psum groups x 512)
BLK = 512           # moving-operand tokens per matmul

f32 = mybir.dt.float32
f16 = mybir.dt.float16


# ----------------------------------------------------------------- planner --

def _greedy_assign(tiles_e, inventory):
    inv = dict(inventory)
    sizes = sorted(inv.keys(), reverse=True)
    order = sorted(range(len(tiles_e)), key=lambda e: -tiles_e[e])
    out = []
    for e in order:
        rem = tiles_e[e]
        toff = 0
        while rem > 0:
            pick = None
            for s in sizes:
                if inv.get(s, 0) > 0 and s <= rem:
                    pick = s
                    break
            if pick is None:
                cands = [s for s in sizes if inv.get(s, 0) > 0 and s >= rem]
                if not cands:
                    return None
                pick = min(cands)
            take = min(rem, pick)
            inv[pick] -= 1
            out.append((e, toff, pick, take))
            rem -= take
            toff += take
    return out


def _distribute_to_cores(P, assignments, n_cores):
    from collections import defaultdict
    P_desc = sorted(P, reverse=True)
    core_slots = []
    for c in range(n_cores):
        d = defaultdict(list)
        for idx, p in enumerate(P_desc):
            d[p].append(idx)
        core_slots.append(d)
    plan = [[None] * len(P_desc) for _ in range(n_cores)]
    rr = {p: 0 for p in set(P_desc)}
    for (e, toff, size, take) in sorted(assignments, key=lambda a: -a[2]):
        start = rr[size]
        for k in range(n_cores):
            c = (start + k) % n_cores
            if core_slots[c][size]:
                idx = core_slots[c][size].pop(0)
                plan[c][idx] = (e, toff, take)
                rr[size] = (c + 1) % n_cores
                break
        else:
            raise AssertionError("inventory accounting bug")
    return P_desc, plan


def make_plan(counts, n_cores=N_CORES, tile=TILE, max_slots=18):
    """Returns (P_desc, plan): P_desc = slot sizes (tiles) desc, shared by all
    cores; plan[c][s] = (expert, tok_offset, n_tokens) with n_tokens possibly 0."""
    counts = np.asarray(counts, dtype=np.int64)
    E = len(counts)
    offsets = np.concatenate([[0], np.cumsum(counts)])
    tiles_e = [max(0, int(np.ceil(c / tile))) for c in counts]
    total = max(1, sum(tiles_e))
    lo = int(np.ceil(total / n_cores))

    size_menu = [64, 48, 40, 32, 24, 20, 16, 12, 8, 6, 4, 3, 2, 1]
    best = None
    rng = np.random.default_rng(0)
    for _ in range(4000):
        C_target = lo + int(rng.integers(0, 6))
        P = []
        rem = C_target
        for s in size_menu:
            if rem <= 0:
                break
            if s > rem:
                continue
            max_n = rem // s
            n = int(rng.integers(0, (max_n if s > 4 else min(max_n, 4)) + 1))
            if len(P) + n > max_slots:
                n = max_slots - len(P)
            P += [s] * n
            rem -= n * s
        while rem > 0 and len(P) < max_slots:
            s = max(x for x in size_menu if x <= rem)
            P.append(s)
            rem -= s
        if rem != 0 or not P:
            continue
        inv = {}
        for p in P:
            inv[p] = inv.get(p, 0) + n_cores
        a = _greedy_assign(tiles_e, inv)
        if a is None:
            continue
        cost = sum(P) + 0.5 * len(P)
        if best is None or cost < best[0]:
            best = (cost, P, a)
    assert best is not None, "no feasible slot profile found"
    _, P, assignments = best
    P_desc, plan_t = _distribute_to_cores(P, assignments, n_cores)

    plan = []
    for c in range(n_cores):
        entries = []
        for piece in plan_t[c]:
            if piece is None:
                entries.append((0, 0, 0))
            else:
                e, toff, t = piece
                tok0 = int(offsets[e]) + toff * tile
                ntok = max(0, min(int(counts[e]) - toff * tile, t * tile))
                entries.append((e, tok0, ntok))
        plan.append(entries)
    return P_desc, plan


# ------------------------------------------------------------ device program --

_program_cache = {}


def build_program(P):
    """Uniform SPMD program for slot profile P (list of tile counts, desc)."""
    key = tuple(P)
    if key in _program_cache:
        return _program_cache[key]

    S = len(P)
    C = sum(P)
    CT = C * TILE

    nc = bacc.Bacc()
    xt = nc.declare_dram_parameter("xt", [D_IN, CT], f16, isOutput=False)
    ws = nc.declare_dram_parameter("ws", [S, D_IN, D_OUT], f16, isOutput=False)
    bs = nc.declare_dram_parameter("bs", [128, S * DC], f32, isOutput=False)
    out = nc.declare_dram_parameter("out", [D_OUT, CT], f32, isOutput=True)

    xt_r = xt.rearrange("(kc p) t -> p kc t", p=128)
    ws_r = ws.rearrange("s (kc p) n -> p s kc n", p=128)
    out_r = out.rearrange("(dc p) t -> p dc t", p=128)

    copy_fn = mybir.ActivationFunctionType.Identity

    with tile.TileContext(nc) as tc:
        with (
            tc.tile_pool(name="xpool", bufs=2) as xpool,
            tc.tile_pool(name="wpool", bufs=2) as wpool,
            tc.tile_pool(name="opool", bufs=3) as opool,
            tc.tile_pool(name="bpool", bufs=1) as bpool,
            tc.tile_pool(name="psum", bufs=2, space="PSUM") as psum,
        ):
            b_sb = bpool.tile([128, S * DC], f32)
            nc.sync.dma_start(b_sb[:], bs[:])

            col = 0  # running token-column base
            for s in range(S):
                w_sb = wpool.tile([128, KC * D_OUT], f16, tag="wsb")
                for kc in range(KC):
                    nc.sync.dma_start(
                        w_sb[:, kc * D_OUT:(kc + 1) * D_OUT], ws_r[:, s, kc, :]
                    )
                slot_tok = P[s] * TILE
                t0 = 0
                while t0 < slot_tok:
                    mtok = min(MEGA_TOK, slot_tok - t0)
                    nblk = (mtok + BLK - 1) // BLK
                    c0 = col + t0
                    x_sb = xpool.tile([128, KC * MEGA_TOK], f16, tag="xsb")
                    for kc in range(KC):
                        nc.sync.dma_start(
                            x_sb[:, kc * MEGA_TOK: kc * MEGA_TOK + mtok],
                            xt_r[:, kc, c0:c0 + mtok],
                        )
                    for dc in range(DC):
                        o_sb = opool.tile([128, MEGA_TOK], f32, tag="osb")
                        acc = psum.tile([128, 4, BLK], f32, name="acc")
                        for kc in range(KC):
                            lhsT = w_sb[:, kc * D_OUT + dc * 128: kc * D_OUT + (dc + 1) * 128]
                            for g in range(nblk):
                                ntok = min(BLK, mtok - g * BLK)
                                nc.tensor.matmul(
                                    acc[:, g, :ntok],
                                    lhsT,
                                    x_sb[:, kc * MEGA_TOK + g * BLK: kc * MEGA_TOK + g * BLK + ntok],
                                    start=(kc == 0),
                                    stop=(kc == KC - 1),
                                )
                        for g in range(nblk):
                            ntok = min(BLK, mtok - g * BLK)
                            nc.scalar.activation(
                                o_sb[:, g * BLK: g * BLK + ntok],
                                acc[:, g, :ntok],
                                copy_fn,
                                bias=b_sb[:, s * DC + dc: s * DC + dc + 1],
                            )
                        nc.gpsimd.dma_start(
                            out_r[:, dc, c0:c0 + mtok], o_sb[:, :mtok]
                        )
                    t0 += mtok
                col += slot_tok
    nc.finalize()
    _program_cache[key] = nc
    return nc


# ------------------------------------------------------------------ kernel --

def kernel(input, expert_frequency, weight, bias):
    input = np.asarray(input)
    counts = np.asarray(expert_frequency)
    weight = np.asarray(weight)
    bias = np.asarray(bias)
    T = input.shape[0]
    in_dtype = input.dtype

    P, plan = make_plan(counts)
    S = len(P)
    C = sum(P)
    CT = C * TILE

    nc = build_program(P)

    # host data prep
    x16t = np.ascontiguousarray(input.T.astype(np.float16))          # [D_IN, T]
    w16t = np.ascontiguousarray(
        weight.transpose(0, 2, 1).astype(np.float16))                # [E, D_IN, D_OUT]
    bias32 = bias.astype(np.float32)

    in_maps = []
    for c in range(N_CORES):
        xt_c = np.zeros((D_IN, CT), np.float16)
        ws_c = np.empty((S, D_IN, D_OUT), np.float16)
        bs_c = np.zeros((128, S * DC), np.float32)
        col = 0
        for s, (e, tok0, ntok) in enumerate(plan[c]):
            if ntok > 0:
                xt_c[:, col:col + ntok] = x16t[:, tok0:tok0 + ntok]
            ws_c[s] = w16t[e]
            bs_c[:, s * DC:(s + 1) * DC] = bias32[e].reshape(DC, 128).T
            col += P[s] * TILE
        in_maps.append({"xt": xt_c, "ws": ws_c, "bs": bs_c})

    res = run_bass_kernel_spmd(nc, in_maps, core_ids=list(range(N_CORES)))

    out_full = np.empty((T, D_OUT), np.float32)
    for c in range(N_CORES):
        oc = res.results[c]["out"]          # [D_OUT, CT]
        col = 0
        for s, (e, tok0, ntok) in enumerate(plan[c]):
            if ntok > 0:
                out_full[tok0:tok0 + ntok, :] = oc[:, col:col + ntok].T
            col += P[s] * TILE
    return out_full.astype(in_dtype, copy=False)
